# revision 2
# baseline (speedup 1.0000x reference)
"""KAN layer (LayerNorm -> per-bin Bernstein spline -> reduce over input dim)
as a Bass/Tile kernel for 8 trn2 NeuronCores. v2.

Sharding: data-parallel over batch (8 rows per core). Poly table rearranged on
host to rows R[i*GRID+g] holding the (k, o) coefficient block of one (input,
bin) pair; row layout [ (4-ni) k-slices bf16 | ni k-slices int8 ], int8 with
one global scale folded into the Bernstein coefficients (ni=0 -> pure bf16).

Per core (8 batch rows b):
  ln:       xn = LN(x) in [8,512] layout; u = ((clip(xn)+1)*0.5)*GRID
  bins:     u transposed via PE to uT [128 i, 32 (c,b)]; floor/frac/Bernstein
            computed there (free dim 32 -> cheap DVE ops)
  gather:   ONE indirect DMA per b: offsets [128, 4] -> G[128, 4*RB] bytes
  contract: 16 matmuls per b; b%4 picks the PE column-group (tile_position)
            so adjacent b's matmuls run concurrently on distinct 32-col strips
  out:      PSUM rows {0,32,64,96} of 2 banks -> OUT sbuf -> y8
"""

import numpy as np

import concourse.bass as bass
import concourse.mybir as mybir
import concourse.tile as tile
from concourse import bacc
from concourse.bass_utils import run_bass_kernel_spmd
from concourse.masks import make_identity

B = 64          # total batch
D_IN = 512
D_OUT = 512
DEG = 3
GRID = 100
GRID_EPS = 1e-6
LN_EPS = 1e-5
N_CORES = 8
BPC = B // N_CORES          # batch rows per core (8)
NROWS = D_IN * GRID         # 51200 gatherable rows
NCH = D_IN // 128           # 4 i-chunks of 128
NK = DEG + 1                # 4 coefficients
TW = NCH * BPC              # 32: transposed-domain free width (c-major, b-minor)

# --- tunables -------------------------------------------------------------
NI8 = 0          # how many of the 4 k-slices are stored int8 (rest bf16)
NWAY = 2         # col-group interleave width (1, 2 or 4 concurrent b's)
DVE_CHUNKS = 2   # dequant: chunks [0, DVE_CHUNKS) on DVE, rest on ACT
MULTI_GATHER = False  # [128,>1] offset gathers broken on HW (one offset/part)
# --------------------------------------------------------------------------

F32 = mybir.dt.float32
I32 = mybir.dt.int32
I8 = mybir.dt.int8
BF16 = mybir.dt.bfloat16
AX = mybir.AxisListType
OP = mybir.AluOpType
AF = mybir.ActivationFunctionType

_CACHE = {}


def _row_bytes(ni):
    return (NK - ni) * 2 * D_OUT + ni * D_OUT


def _build_nc(Mscale, apply_affine, ni, nway, loop_n=1, use_tp=True,
              multi_gather=MULTI_GATHER, repeat=1):
    """Mscale: per-k dequant scale folded into basis (1.0 for bf16 k's)."""
    nb = NK - ni
    RB = _row_bytes(ni)
    nc = bacc.Bacc("TRN2", target_bir_lowering=False, debug=False)

    x8 = nc.declare_dram_parameter("x8", [BPC, D_IN], F32, isOutput=False)
    R = nc.declare_dram_parameter("R", [NROWS, RB], I8, isOutput=False)
    if apply_affine:
        w8 = nc.declare_dram_parameter("w8", [BPC, D_IN], F32, isOutput=False)
        b8 = nc.declare_dram_parameter("b8", [BPC, D_IN], F32, isOutput=False)
    y8 = nc.declare_dram_parameter("y8", [BPC, D_OUT], F32, isOutput=True)

    with tile.TileContext(nc) as tc:
        with (
            tc.tile_pool(name="const", bufs=1) as cp,
            tc.tile_pool(name="work", bufs=1) as wp,
            tc.tile_pool(name="gpool", bufs=4) as gp,
            tc.tile_pool(name="dqpool", bufs=4) as dq,
            tc.tile_pool(name="outp", bufs=2) as op_,
            tc.tile_pool(name="ptr", bufs=2, space="PSUM") as ptr,
            tc.tile_pool(name="pacc", bufs=2, space="PSUM") as pacc,
        ):
            ident = cp.tile([128, 128], F32, tag="ident")
            make_identity(nc, ident[:])

            # iota32[p, c*8+b] = 100*p + 12800*c  (row base i*GRID, i=c*128+p)
            iota32 = cp.tile([128, TW], I32, tag="iota32")
            nc.gpsimd.iota(iota32[:], pattern=[[GRID * 128, NCH], [0, BPC]],
                           base=0, channel_multiplier=GRID)

            x = cp.tile([BPC, D_IN], F32, tag="x")
            nc.sync.dma_start(x[:], x8[:])
            if apply_affine:
                wt = cp.tile([BPC, D_IN], F32, tag="wt")
                bt = cp.tile([BPC, D_IN], F32, tag="bt")
                nc.sync.dma_start(wt[:], w8[:])
                nc.sync.dma_start(bt[:], b8[:])

            def body():
                # ---- LayerNorm in [8, 512] (two-pass, matches jnp) ----
                sumx = wp.tile([BPC, 1], F32, tag="sumx")
                nc.vector.tensor_reduce(sumx[:], x[:], axis=AX.X, op=OP.add)
                mean = wp.tile([BPC, 1], F32, tag="mean")
                nc.vector.tensor_scalar_mul(mean[:], sumx[:], 1.0 / D_IN)
                xc = wp.tile([BPC, D_IN], F32, tag="xc")
                nc.vector.tensor_scalar(xc[:], x[:], mean[:, :1], None,
                                        OP.subtract)
                sq = wp.tile([BPC, D_IN], F32, tag="sq")
                nc.scalar.square(sq[:], xc[:])
                v = wp.tile([BPC, 1], F32, tag="v")
                nc.vector.tensor_reduce(v[:], sq[:], axis=AX.X, op=OP.add)
                nc.vector.tensor_scalar(v[:], v[:], 1.0 / D_IN, LN_EPS,
                                        OP.mult, OP.add)
                s = wp.tile([BPC, 1], F32, tag="s")
                nc.scalar.sqrt(s[:], v[:])
                r0 = wp.tile([BPC, 1], F32, tag="r0")
                nc.vector.reciprocal(r0[:], s[:])
                r2 = wp.tile([BPC, 1], F32, tag="r2")
                nc.vector.tensor_tensor(out=r2[:], in0=r0[:], in1=r0[:],
                                        op=OP.mult)
                nc.vector.tensor_tensor(out=r2[:], in0=r2[:], in1=v[:],
                                        op=OP.mult)
                nc.vector.tensor_scalar(r2[:], r2[:], -0.5, 1.5, OP.mult,
                                        OP.add)
                rstd = wp.tile([BPC, 1], F32, tag="rstd")
                nc.vector.tensor_tensor(out=rstd[:], in0=r0[:], in1=r2[:],
                                        op=OP.mult)

                xn = wp.tile([BPC, D_IN], F32, tag="xn")
                nc.vector.tensor_scalar(xn[:], xc[:], rstd[:, :1], None,
                                        OP.mult)
                if apply_affine:
                    nc.vector.tensor_tensor(out=xn[:], in0=xn[:], in1=wt[:],
                                            op=OP.mult)
                    nc.vector.tensor_tensor(out=xn[:], in0=xn[:], in1=bt[:],
                                            op=OP.add)

                # clip, map to [0, GRID) -- same op order as the reference
                cl = wp.tile([BPC, D_IN], F32, tag="cl")
                nc.vector.tensor_scalar(cl[:], xn[:], -1.0 + GRID_EPS,
                                        1.0 - GRID_EPS, OP.max, OP.min)
                u = wp.tile([BPC, D_IN], F32, tag="u")
                nc.vector.tensor_scalar(u[:], cl[:], 1.0, 0.5, OP.add, OP.mult)
                nc.vector.tensor_scalar(u[:], u[:], float(GRID), None, OP.mult)

                # ---- transpose u -> uT [128, 32] (c-major, b-minor) ----
                pt = ptr.tile([128, TW], F32, tag="pt")
                for c in range(NCH):
                    nc.tensor.transpose(pt[:, c * BPC:(c + 1) * BPC],
                                        u[:, c * 128:(c + 1) * 128],
                                        ident[:BPC, :BPC])
                uT = wp.tile([128, TW], F32, tag="uT")
                nc.vector.tensor_copy(uT[:], pt[:])

                # ---- floor/frac in [128, 32] ----
                i1 = wp.tile([128, TW], I32, tag="i1")
                nc.vector.tensor_copy(i1[:], uT[:])
                f1 = wp.tile([128, TW], F32, tag="f1")
                nc.vector.tensor_copy(f1[:], i1[:])
                gt = wp.tile([128, TW], F32, tag="gt")
                nc.vector.tensor_tensor(out=gt[:], in0=f1[:], in1=uT[:],
                                        op=OP.is_gt)
                flr = wp.tile([128, TW], F32, tag="flr")
                nc.vector.tensor_tensor(out=flr[:], in0=f1[:], in1=gt[:],
                                        op=OP.subtract)
                tT = wp.tile([128, TW], F32, tag="tT")
                nc.vector.tensor_tensor(out=tT[:], in0=uT[:], in1=flr[:],
                                        op=OP.subtract)
                iflr = wp.tile([128, TW], I32, tag="iflr")
                nc.vector.tensor_copy(iflr[:], flr[:])
                offs = wp.tile([128, TW], I32, tag="offs")
                nc.vector.tensor_tensor(out=offs[:], in0=iflr[:],
                                        in1=iota32[:], op=OP.add)
                # b-major copy: DMA offset APs must be contiguous per b
                offsBM = wp.tile([128, TW], I32, tag="offsBM")
                nc.vector.tensor_copy(
                    offsBM[:].rearrange("p (b c) -> p c b", c=NCH),
                    offs[:].rearrange("p (c b) -> p c b", c=NCH))
                offsB = offsBM[:].rearrange("p (b c) -> p b c", b=BPC)

                # ---- Bernstein basis in [128, 32] ----
                # B0=(1-t)^3 B1=3t(1-t)^2 B2=3t^2(1-t) B3=t^3
                sm = wp.tile([128, TW], F32, tag="sm")
                nc.vector.tensor_scalar(sm[:], tT[:], -1.0, 1.0, OP.mult,
                                        OP.add)
                t2 = wp.tile([128, TW], F32, tag="t2")
                nc.vector.tensor_tensor(out=t2[:], in0=tT[:], in1=tT[:],
                                        op=OP.mult)
                s2 = wp.tile([128, TW], F32, tag="s2")
                nc.vector.tensor_tensor(out=s2[:], in0=sm[:], in1=sm[:],
                                        op=OP.mult)
                Bf = [wp.tile([128, TW], F32, tag=f"Bf{k}", name=f"Bf{k}")
                      for k in range(NK)]
                nc.vector.tensor_tensor(out=Bf[0][:], in0=s2[:], in1=sm[:],
                                        op=OP.mult)
                nc.vector.tensor_tensor(out=Bf[3][:], in0=t2[:], in1=tT[:],
                                        op=OP.mult)
                u1 = wp.tile([128, TW], F32, tag="u1")
                nc.vector.tensor_tensor(out=u1[:], in0=tT[:], in1=s2[:],
                                        op=OP.mult)
                nc.vector.tensor_scalar_mul(Bf[1][:], u1[:], 3.0)
                u2 = wp.tile([128, TW], F32, tag="u2")
                nc.vector.tensor_tensor(out=u2[:], in0=t2[:], in1=sm[:],
                                        op=OP.mult)
                nc.vector.tensor_scalar_mul(Bf[2][:], u2[:], 3.0)

                bvT = []
                for k in range(NK):
                    bk = wp.tile([128, TW], BF16, tag=f"bvT{k}")
                    nc.scalar.activation(bk[:], Bf[k][:], AF.Copy,
                                         scale=Mscale[k])
                    bvT.append(bk)

                # ---- gather + (dequant) + contract ----
                Gs = [None] * BPC
                Ds = [None] * BPC

                def issue_gather(b):
                    if multi_gather:
                        G = gp.tile([128, NCH * RB], I8, tag="G")
                        nc.gpsimd.indirect_dma_start(
                            out=G[:], out_offset=None, in_=R[:],
                            in_offset=bass.IndirectOffsetOnAxis(
                                ap=offsB[:, b, :], axis=0))
                    else:
                        G = gp.tile([128, NCH * RB], I8, tag="G")
                        gv = G[:].rearrange("p (c r) -> p c r", c=NCH)
                        for c in range(NCH):
                            nc.gpsimd.indirect_dma_start(
                                out=gv[:, c, :], out_offset=None, in_=R[:],
                                in_offset=bass.IndirectOffsetOnAxis(
                                    ap=offs[:, c * BPC + b:c * BPC + b + 1],
                                    axis=0))
                    Gs[b] = G
                    if ni:
                        D = dq.tile([128, NCH * ni * D_OUT], BF16, tag="D")
                        gv = G[:].rearrange("p (c r) -> p c r", c=NCH)
                        dv = D[:].rearrange("p (c r) -> p c r", c=NCH)
                        if DVE_CHUNKS:
                            nc.vector.tensor_copy(
                                dv[:, :DVE_CHUNKS, :],
                                gv[:, :DVE_CHUNKS, nb * 2 * D_OUT:])
                        if DVE_CHUNKS < NCH:
                            nc.scalar.copy(
                                dv[:, DVE_CHUNKS:, :],
                                gv[:, DVE_CHUNKS:, nb * 2 * D_OUT:])
                        Ds[b] = D

                def rhs_ap(b, c, k):
                    if k < nb:
                        off = c * RB + k * 2 * D_OUT
                        return Gs[b][:, off:off + 2 * D_OUT].bitcast(BF16)
                    off = (c * ni + (k - nb)) * D_OUT
                    return Ds[b][:, off:off + D_OUT]

                if use_tp:
                  for _rep in range(repeat):
                    OUT = op_.tile([128, 2 * D_OUT], F32, tag="OUT")
                    for blk in range(BPC // 4):
                        acc = pacc.tile([128, D_OUT], F32, tag=f"acc{blk % 2}")
                        for g in range(4 // nway):
                            bs = [blk * 4 + g * nway + i for i in range(nway)]
                            for b in bs:
                                issue_gather(b)
                            for c in range(NCH):
                                for k in range(NK):
                                    for b in bs:
                                        j = b % 4
                                        nc.tensor.matmul(
                                            acc[32 * j:32 * j + 1, :],
                                            lhsT=bvT[k][:, c * BPC + b:
                                                        c * BPC + b + 1],
                                            rhs=rhs_ap(b, c, k),
                                            start=(c == 0 and k == 0),
                                            stop=(c == NCH - 1 and
                                                  k == NK - 1),
                                            tile_position=(0, 32 * j),
                                        )
                            for b in bs:
                                j = b % 4
                                src = acc[32 * j:32 * j + 1, :]
                                dst = OUT[32 * j:32 * j + 1,
                                          blk * D_OUT:(blk + 1) * D_OUT]
                                if b % 2 == 0:
                                    nc.vector.tensor_copy(dst, src)
                                else:
                                    nc.scalar.copy(dst, src)
                    OUTv = OUT[:].rearrange("(a q) (h f) -> q a h f",
                                            q=32, h=2)
                    nc.sync.dma_start(y8[0:4, :], OUTv[0, :, 0, :])
                    nc.sync.dma_start(y8[4:8, :], OUTv[0, :, 1, :])
                else:
                  for _rep in range(repeat):
                    OUT = op_.tile([1, BPC * D_OUT], F32, tag="OUT")
                    for b in range(BPC):
                        issue_gather(b)
                        acc = pacc.tile([1, D_OUT], F32, tag="acc")
                        for c in range(NCH):
                            for k in range(NK):
                                nc.tensor.matmul(
                                    acc[:],
                                    lhsT=bvT[k][:, c * BPC + b:
                                                c * BPC + b + 1],
                                    rhs=rhs_ap(b, c, k),
                                    start=(c == 0 and k == 0),
                                    stop=(c == NCH - 1 and k == NK - 1),
                                )
                        dst = OUT[0:1, b * D_OUT:(b + 1) * D_OUT]
                        if b % 2 == 0:
                            nc.vector.tensor_copy(dst, acc[:])
                        else:
                            nc.scalar.copy(dst, acc[:])
                    nc.sync.dma_start(y8[:], OUT[0:1, :])

            if loop_n > 1:
                with tc.For_i(0, loop_n):
                    body()
            else:
                body()

    nc.compile()
    return nc


def _prep_R(poly_matrix, scale, ni):
    """poly[i, o, g, k] -> rows R[i*GRID+g] = [bf16 k<nb | int8 k>=nb]."""
    import ml_dtypes
    nb = NK - ni
    P = np.transpose(np.asarray(poly_matrix, np.float32), (0, 2, 3, 1))
    # P now [i, g, k, o]
    parts = []
    if nb:
        pb = np.ascontiguousarray(P[:, :, :nb, :]).astype(ml_dtypes.bfloat16)
        parts.append(pb.reshape(NROWS, nb * D_OUT).view(np.uint8))
    if ni:
        pi = np.clip(np.round(
            np.ascontiguousarray(P[:, :, nb:, :]) / scale), -127, 127
        ).astype(np.int8)
        parts.append(pi.reshape(NROWS, ni * D_OUT).view(np.uint8))
    Rb = np.concatenate(parts, axis=1) if len(parts) > 1 else parts[0]
    assert Rb.shape == (NROWS, _row_bytes(ni))
    return np.ascontiguousarray(Rb).view(np.int8)


def get_compiled(basis_matrix, ln_weight, ln_bias, scale, ni=NI8, nway=NWAY,
                 loop_n=1, use_tp=True, multi_gather=MULTI_GATHER, repeat=1):
    apply_affine = not (np.all(ln_weight == 1.0) and np.all(ln_bias == 0.0))
    key = (apply_affine, float(scale), ni, nway, loop_n, use_tp, multi_gather,
           repeat)
    if key not in _CACHE:
        Mscale = [float(scale) if k >= NK - ni else 1.0 for k in range(NK)]
        _CACHE[key] = _build_nc(Mscale, apply_affine, ni, nway, loop_n,
                                use_tp, multi_gather, repeat)
    return _CACHE[key], apply_affine


def make_in_maps(x, poly_matrix, ln_weight, ln_bias, apply_affine, scale,
                 ni=NI8):
    Rb = _prep_R(np.asarray(poly_matrix), scale, ni)
    x = np.asarray(x, np.float32)
    maps = []
    for c in range(N_CORES):
        m = {"x8": np.ascontiguousarray(x[c * BPC:(c + 1) * BPC]), "R": Rb}
        if apply_affine:
            m["w8"] = np.ascontiguousarray(np.broadcast_to(
                np.asarray(ln_weight, np.float32), (BPC, D_IN)))
            m["b8"] = np.ascontiguousarray(np.broadcast_to(
                np.asarray(ln_bias, np.float32), (BPC, D_IN)))
        maps.append(m)
    return maps


def kernel(x, poly_matrix, ln_weight, ln_bias, basis_matrix):
    scale = float(np.abs(np.asarray(poly_matrix)).max()) / 127.0
    nc, apply_affine = get_compiled(basis_matrix, ln_weight, ln_bias, scale)
    in_maps = make_in_maps(x, poly_matrix, ln_weight, ln_bias, apply_affine,
                           scale)
    res = run_bass_kernel_spmd(nc, in_maps, core_ids=list(range(N_CORES)))
    y = np.concatenate([res.results[c]["y8"] for c in range(N_CORES)], axis=0)
    return y.astype(np.float32)
